# revision 27
# baseline (speedup 1.0000x reference)
"""Trainium2 Bass kernel for AnnealingTopKSoftMax (top-8 masked softmax).

Computes, for each row of a [131072, 512] f32 tensor:
  out = softmax(where(mask_top8(x), x, -1e16))
which equals: exp(x)/sum(exp(top8(x))) at the top-8 positions, 0 elsewhere.

Strategy (pure data parallelism, batch axis sharded over 8 NeuronCores).
Per [128, 8, 512] block (rows on partitions, 8 row-subtiles per partition),
per-subtile granularity so all five engines pipeline freely:
  v8[c]  = max8(x_c)                      # DVE: 8 largest per row (desc)
  psum_c = exp(x_c)                       # ACT writes e into a PSUM bank
  e8     = exp(v8);  r8 = 1/sum(e8)       # tiny per-row denominators
  z_c    = match_replace(psum_c, e8[c], 0)  # DVE reads PSUM, writes SBUF
  psum_c += (-I) @ z_c                    # TensorE accumulate -> e - z
  out_c  = psum_c * r8[c]                 # ACT readback * 1/s
match_replace replaces exactly one occurrence per needle (first match),
reproducing jax.lax.top_k's lowest-index tie-breaking exactly (exp is
injective over the top-8 value range for this data; verified bitwise).
"""

import os
import sys
import types

import numpy as np

import concourse.bacc as bacc
import concourse.tile as tile
from concourse import mybir
from concourse.bass_utils import run_bass_kernel_spmd
from concourse.masks import make_identity


def _install_ntff_hook() -> bool:
    """Provide antenv.axon_hooks (absent in this container) so
    run_bass_kernel_spmd(trace=True) can capture NTFF profiles under axon."""
    try:
        from antenv.axon_hooks import get_axon_ntff_profile_hook  # noqa: F401

        return True
    except ImportError:
        pass
    try:
        import antenv
        from trn_agent_boot.trn_boot import _ntff_profile_via_ctypes

        hook = _ntff_profile_via_ctypes("/opt/axon/libaxon_pjrt.so")
        mod = types.ModuleType("antenv.axon_hooks")
        _h = [hook]
        mod.set_axon_ntff_profile_hook = lambda h: _h.__setitem__(0, h)
        mod.get_axon_ntff_profile_hook = lambda: _h[0]
        sys.modules["antenv.axon_hooks"] = mod
        antenv.axon_hooks = mod
        return hook is not None
    except Exception:
        return False


N_CORES = 8
BATCH = 131072
DEPTH = 512
ROWS_PER_CORE = BATCH // N_CORES  # 16384
P = 128          # SBUF partitions; rows per sub-tile
C = 8            # row-subtiles per partition per block (16KB contiguous DMA)
BLOCK_ROWS = P * C               # 1024
N_BLOCKS = ROWS_PER_CORE // BLOCK_ROWS  # 16

F32 = mybir.dt.float32
Exp = mybir.ActivationFunctionType.Exp
Copy = mybir.ActivationFunctionType.Copy


def _build(n_blocks: int = N_BLOCKS):
    rows = n_blocks * BLOCK_ROWS
    nc = bacc.Bacc(
        "TRN2", target_bir_lowering=False, debug=False, num_devices=N_CORES
    )
    x = nc.dram_tensor("x", [rows, DEPTH], F32, kind="ExternalInput")
    out = nc.dram_tensor("out", [rows, DEPTH], F32, kind="ExternalOutput")

    # row = n*1024 + p*8 + c  ->  partition p holds 8 consecutive rows per block
    xv = x.ap().rearrange("(n p c) d -> p n c d", p=P, c=C)
    ov = out.ap().rearrange("(n p c) d -> p n c d", p=P, c=C)

    with tile.TileContext(nc) as tc:
        with (
            tc.tile_pool(name="consts", bufs=1) as consts,
            tc.tile_pool(name="xs", bufs=4) as xs_pool,
            tc.tile_pool(name="zs", bufs=4) as zs_pool,
            tc.tile_pool(name="stats", bufs=4) as st_pool,
            tc.tile_pool(name="psum", bufs=8, space="PSUM") as ps_pool,
        ):
            nident = consts.tile([P, P], F32)
            make_identity(nc, nident[:])
            nc.vector.tensor_scalar_mul(nident[:], nident[:], -1.0)

            pending = None

            def phase1(n):
                """DMA in + find (max8) + denominators (with sign folded)."""
                xt = xs_pool.tile([P, C, DEPTH], F32)
                v8 = st_pool.tile([P, C, 8], F32)
                e8 = st_pool.tile([P, C, 8], F32)
                s8 = st_pool.tile([P, C], F32)
                r8 = st_pool.tile([P, C], F32)
                nc.sync.dma_start(out=xt[:], in_=xv[:, n, :, :])
                for c in range(C):
                    nc.vector.max(out=v8[:, c, :], in_=xt[:, c, :])
                nc.scalar.activation(
                    out=e8.rearrange("p c k -> p (c k)"),
                    in_=v8.rearrange("p c k -> p (c k)"),
                    func=Exp,
                )
                nc.vector.tensor_reduce(
                    out=s8[:],
                    in_=e8[:],
                    axis=mybir.AxisListType.X,
                    op=mybir.AluOpType.add,
                )
                nc.vector.reciprocal(out=r8[:], in_=s8[:])
                return (n, xt, e8, r8)

            def phase2(state):
                """exp into PSUM -> locate -> -z accumulate -> readback."""
                n, xt, e8, r8 = state
                zt = zs_pool.tile([P, C, DEPTH], F32)
                pts = []
                for c in range(C):
                    pt = ps_pool.tile([P, DEPTH], F32)
                    pts.append(pt)
                    nc.scalar.activation(
                        out=pt[:], in_=xt[:, c, :], func=Exp
                    )
                for c in range(C):
                    nc.vector.match_replace(
                        out=zt[:, c, :],
                        in_to_replace=e8[:, c, :],
                        in_values=pts[c][:],
                        imm_value=0.0,
                    )
                for c in range(C):
                    nc.tensor.matmul(
                        pts[c][:],
                        nident[:],
                        zt[:, c, :],
                        start=False,
                        stop=True,
                        skip_group_check=True,
                    )
                for c in range(C - 1):
                    nc.scalar.activation(
                        out=xt[:, c, :],
                        in_=pts[c][:],
                        func=Copy,
                        bias=0.0,
                        scale=r8[:, c : c + 1],
                    )
                # balance: one readback per block on DVE (ACT is the hot engine)
                nc.vector.tensor_scalar(
                    xt[:, C - 1, :],
                    pts[C - 1][:],
                    r8[:, C - 1 : C],
                    None,
                    mybir.AluOpType.mult,
                )
                nc.sync.dma_start(out=ov[:, n, :, :], in_=xt[:])

            # software-pipelined emission: one-block lookahead
            for n in range(n_blocks):
                state = phase1(n)
                if pending is not None:
                    phase2(pending)
                pending = state
            phase2(pending)
    nc.compile()
    return nc


def kernel(**inputs: np.ndarray) -> np.ndarray:
    full = np.ascontiguousarray(inputs["inputs"], dtype=np.float32)
    assert full.shape == (BATCH, DEPTH), full.shape

    nc = _build()
    in_maps = [
        {"x": np.ascontiguousarray(full[i * ROWS_PER_CORE : (i + 1) * ROWS_PER_CORE])}
        for i in range(N_CORES)
    ]
    tr_env = os.environ.get("BASS_TRACE", "")
    trace = tr_env not in ("", "0", "false", "False")
    if trace:
        trace = _install_ntff_hook()
    try:
        res = run_bass_kernel_spmd(
            nc, in_maps, core_ids=list(range(N_CORES)), trace=trace
        )
    except Exception:
        if not trace:
            raise
        os.environ["BASS_NEVER_TRACE"] = "1"
        try:
            res = run_bass_kernel_spmd(
                nc, in_maps, core_ids=list(range(N_CORES)), trace=False
            )
        finally:
            os.environ.pop("BASS_NEVER_TRACE", None)
    kernel.last_result = res
    return np.concatenate([r["out"] for r in res.results], axis=0)
